# revision 42
# baseline (speedup 1.0000x reference)
"""Trainium2 Bass kernel for Performer-style (FAVOR+) causal linear attention, v2.

Per (b,h): qp = relu(q @ ptT/sqrt(M)), kp likewise; causal chunked linear
attention, chunk C=128, groups of G=2 chunks, superblocks of 2 groups.

v2 layout/structure vs v1 (74.7us vs 81.4us measured):
  - Features per 256-token group: 4 N=256 matmuls with q-features on PE rows
    0:63 and k-features on rows 64:127 (base_partition row tiling ->
    concurrent pairs), emitted ONE GROUP AHEAD (software pipeline) so the
    relu eviction never gates the A-matmuls; one merged relu ACTIVATE
    [128, 1024] on ACT evicts all four slices (amortizes the 352-cyc per-op
    overhead and keeps the A-matmul dependency off the congested DVE queue).
  - num/den accumulate TRANSPOSED: psN^T[d, i] (65 partitions) so the
    stationary operands of the num matmuls are 65-column tiles (vaug chunks,
    KV snapshot slices) -> LDWEIGHTS 54ns instead of 113ns.
  - psA layout [diag0 | cross | diag1]; only the two diagonal blocks get the
    triangular mask (one strided 3D-AP DVE tensor_tensor); the cross block is
    a plain DVE convert-copy (it is multiplied by ones in v1).
  - psN evicted once per superblock [65, 512] (ACT), DMA'd straight out
    per superblock (no tail batch).
  - KV state in PSUM fp32 as in v1; snapshot once per group via one strided
    DVE copy; the last group's dead KV update/psP/snapshot are skipped.
Sharding: B*H = 16 pairs, 2 per core on 8 cores; division on host.
"""

import os
import sys

import numpy as np

sys.path.insert(0, "/opt/trn_rl_repo")

B, L, H, D, M = 2, 4096, 8, 64, 256
C = 128             # chunk length
NCHUNK = L // C     # 32
G = 2               # chunks per group
NGROUP = NCHUNK // G            # 16
NSUPER = NGROUP // 2            # 8 superblocks of 512 tokens
PAIRS_PER_CORE = 2
N_CORES = 8
VW = 72             # padded v_aug chunk width (64 v + 1 ones + 7 pad)
OW = 65             # 64 num + den
KVW = 72            # psKV slice stride
RATIO = 1.0 / np.sqrt(np.float32(M))

_CACHED_NC = None


def _build_program():
    import concourse.tile as tile
    from concourse import bacc, mybir
    from contextlib import ExitStack

    f32 = mybir.dt.float32
    bf16 = mybir.dt.bfloat16
    fp8 = mybir.dt.float8e4

    nc = bacc.Bacc("TRN2", target_bir_lowering=False, debug=False)

    qkt_d = nc.dram_tensor("qkt", [PAIRS_PER_CORE, 128, L], bf16, kind="ExternalInput")
    vaug_d = nc.dram_tensor(
        "vaug", [PAIRS_PER_CORE, 128, NCHUNK * VW], bf16, kind="ExternalInput"
    )
    ptqk_d = nc.dram_tensor("ptqk", [128, M], bf16, kind="ExternalInput")
    ptkz_d = nc.dram_tensor("ptkz", [128, M], bf16, kind="ExternalInput")
    maskd_d = nc.dram_tensor("maskd", [128, 3 * C], bf16, kind="ExternalInput")
    out_d = nc.dram_tensor(
        "out", [PAIRS_PER_CORE, OW, L], f32, kind="ExternalOutput"
    )

    with tile.TileContext(nc) as tc, ExitStack() as ctx:
        const_pool = ctx.enter_context(tc.tile_pool(name="const", bufs=1))
        io_pool = ctx.enter_context(tc.tile_pool(name="io", bufs=2))
        feat_pool = ctx.enter_context(tc.tile_pool(name="feat", bufs=2))
        kp_pool = ctx.enter_context(tc.tile_pool(name="kp", bufs=2))
        am_pool = ctx.enter_context(tc.tile_pool(name="am", bufs=2))
        st_pool = ctx.enter_context(tc.tile_pool(name="st", bufs=3))
        on_pool = ctx.enter_context(tc.tile_pool(name="on", bufs=2))

        ps_qk = ctx.enter_context(tc.tile_pool(name="psqk", bufs=2, space="PSUM"))
        ps_p = ctx.enter_context(tc.tile_pool(name="psp", bufs=1, space="PSUM"))
        ps_a = ctx.enter_context(tc.tile_pool(name="psa", bufs=1, space="PSUM"))
        ps_n = ctx.enter_context(tc.tile_pool(name="psn", bufs=1, space="PSUM"))
        ps_kv = ctx.enter_context(tc.tile_pool(name="pskv", bufs=1, space="PSUM"))

        ptqk_sb = const_pool.tile([128, M], bf16)
        nc.scalar.dma_start(ptqk_sb[:], ptqk_d.ap())
        ptkz_sb = const_pool.tile([128, M], bf16)
        maskd_sb = const_pool.tile([128, 3 * C], bf16)

        # Persistent fp32 KV state for both pairs in one PSUM bank; single
        # accumulation group over the whole kernel, per-byte first-touch.
        psKV_all = ps_kv.tile([128, PAIRS_PER_CORE * 2 * KVW], f32, tag="psKV")

        for bh in range(PAIRS_PER_CORE):
            qkt_sb = io_pool.tile([128, L], bf16, tag="qkt_sb")
            vaug_sb = io_pool.tile([128, NCHUNK * VW], bf16, tag="vaug_sb")
            if bh == 0:
                qsplits = [0, 256, 512, 1024, 2048, 3072, L]
                vsplits = [0, 2 * VW, 8 * VW, 16 * VW, NCHUNK * VW]
            else:
                qsplits = [0, 2048, L]
                vsplits = [0, NCHUNK * VW]
            nc.sync.dma_start(
                qkt_sb[:, 0 : qsplits[1]], qkt_d.ap()[bh, :, 0 : qsplits[1]]
            )
            if bh == 0:
                nc.scalar.dma_start(ptkz_sb[:], ptkz_d.ap())
            (nc.scalar if bh == 0 else nc.sync).dma_start(
                vaug_sb[:, 0 : vsplits[1]], vaug_d.ap()[bh, :, 0 : vsplits[1]]
            )
            if bh == 0:
                nc.sync.dma_start(maskd_sb[:], maskd_d.ap())
            for a, b in zip(qsplits[1:-1], qsplits[2:]):
                nc.sync.dma_start(qkt_sb[:, a:b], qkt_d.ap()[bh, :, a:b])
            for a, b in zip(vsplits[1:-1], vsplits[2:]):
                # keep the sync queue free for qkt chunks during the ramp
                (nc.scalar if bh == 0 else nc.sync).dma_start(
                    vaug_sb[:, a:b], vaug_d.ap()[bh, :, a:b]
                )

            psKV = psKV_all[:, bh * 2 * KVW : (bh + 1) * 2 * KVW]
            prev_snap = None

            def emit_features(g):
                """Feature matmuls + split relu eviction for group g.

                q-features on PE rows 0:63, k-features on rows 64:127
                (concurrent row-groups); relu split DVE (k) / ACT (q).
                Returns the bf16 [128, 1024] tile [q_s0|q_s1|k_s0|k_s1].
                """
                tk = slice(g * 256, (g + 1) * 256)
                psQK = ps_qk.tile([128, 4 * 256], f32, tag="psQK")
                for s in range(2):
                    nc.tensor.matmul(
                        psQK[:, s * 256 : (s + 1) * 256],
                        lhsT=ptqk_sb[0:64, s * 128 : (s + 1) * 128],
                        rhs=qkt_sb[0:64, tk],
                        start=True,
                        stop=True,
                    )
                    nc.tensor.matmul(
                        psQK[:, (2 + s) * 256 : (3 + s) * 256],
                        lhsT=ptqk_sb[64:128, s * 128 : (s + 1) * 128],
                        rhs=qkt_sb[64:128, tk],
                        start=True,
                        stop=True,
                    )
                t = feat_pool.tile([128, 4 * 256], fp8, tag="qkpT")
                nc.scalar.activation(
                    t[:], psQK[:], mybir.ActivationFunctionType.Relu
                )
                return t

            feat_next = emit_features(0)

            for sp in range(NSUPER):
                psN = ps_n.tile([OW, 512], f32, tag="psN")

                for gi in range(2):
                    g = sp * 2 + gi
                    c0 = g * G          # chunk indices
                    toff = 0            # token offset within feature block
                    ncol = gi * 256     # psN half

                    qkpT_sb = feat_next
                    if g < NGROUP - 1:
                        feat_next = emit_features(g + 1)

                    def qpT(s, off, width=C):
                        return qkpT_sb[:, s * 256 + off : s * 256 + off + width]

                    def kpT(s, off, width=C):
                        return qkpT_sb[
                            :, 512 + s * 256 + off : 512 + s * 256 + off + width
                        ]

                    v0 = vaug_sb[:, c0 * VW : c0 * VW + OW]
                    v1 = vaug_sb[:, (c0 + 1) * VW : (c0 + 1) * VW + OW]

                    # ---- kp natural (for KV update): 2 N=256 matmuls; the
                    # last group never uses its KV update -> skip ----
                    if g < NGROUP - 1:
                        psP = ps_p.tile([128, 512], f32, tag="psP")
                        for cc in range(G):
                            nc.tensor.matmul(
                                psP[:, cc * 256 : (cc + 1) * 256],
                                lhsT=qkt_sb[:, (c0 + cc) * C : (c0 + cc + 1) * C],
                                rhs=ptkz_sb[:],
                                start=True,
                                stop=True,
                            )
                        kp_sb = kp_pool.tile([128, 512], bf16, tag="kp_sb")
                        nc.vector.tensor_scalar(
                            kp_sb[:], psP[:], 0.0, None, mybir.AluOpType.max
                        )

                    # ---- A^T blocks [diag0 | cross | diag1]: fp8 DoubleRow,
                    # both feature slices (K=256) packed 2-per-cell ----
                    qpT3 = qkpT_sb[:, 0:512].rearrange("p (s w) -> p s w", s=2)
                    kpT3 = qkpT_sb[:, 512:1024].rearrange("p (s w) -> p s w", s=2)
                    psA = ps_a.tile([128, 3 * C], f32, tag="psA")
                    nc.tensor.matmul(
                        psA[:, 0 : 2 * C],
                        lhsT=kpT3[:, :, 0:C],
                        rhs=qpT3,
                        start=True,
                        stop=True,
                        perf_mode=mybir.MatmulPerfMode.DoubleRow,
                        skip_group_check=True,
                    )
                    # diag1 at FD=128: DoubleRow's weight-load penalty loses
                    # to normal-mode fp8 with FWL -> two K=128 matmuls
                    for s in range(2):
                        nc.tensor.matmul(
                            psA[:, 2 * C : 3 * C],
                            lhsT=qkpT_sb[:, 512 + s * 256 + C : 512 + s * 256 + 2 * C],
                            rhs=qkpT_sb[:, s * 256 + C : s * 256 + 2 * C],
                            start=(s == 0),
                            stop=(s == 1),
                            skip_group_check=True,
                        )
                    amask_sb = am_pool.tile([128, 3 * C], bf16, tag="amask")
                    # one [tri|ones|tri] masked multiply: one DVE op beats
                    # diag-TT + cross-copy (per-op overhead); keeps DVE's
                    # per-group total under the group period
                    nc.vector.tensor_mul(amask_sb[:], psA[:], maskd_sb[:])

                    # ---- num^T (+den row 64): inter first, then masked ----
                    if g > 0:
                        for s in range(2):
                            nc.tensor.matmul(
                                psN[:, ncol : ncol + 256],
                                lhsT=prev_snap[:, s * OW : (s + 1) * OW],
                                rhs=qpT(s, toff, width=256),
                                start=(s == 0),
                                stop=False,
                                skip_group_check=True,
                            )
                    nc.tensor.matmul(
                        psN[:, ncol : ncol + 256],
                        lhsT=v0,
                        rhs=amask_sb[:, 0 : 2 * C],
                        start=(g == 0),
                        stop=False,
                        skip_group_check=True,
                    )
                    nc.tensor.matmul(
                        psN[:, ncol + C : ncol + 2 * C],
                        lhsT=v1,
                        rhs=amask_sb[:, 2 * C : 3 * C],
                        start=False,
                        stop=True,
                        skip_group_check=True,
                    )

                    # ---- KV state accumulation (after snapshot below of the
                    # previous group has read the old state); the last group's
                    # update feeds nothing -> skipped entirely ----
                    if g < NGROUP - 1:
                        for cc in range(G):
                            vsl = vaug_sb[:, (c0 + cc) * VW : (c0 + cc) * VW + OW]
                            for s in range(2):
                                nc.tensor.matmul(
                                    psKV[:, s * KVW : s * KVW + OW],
                                    lhsT=kp_sb[:, cc * 256 + s * 128 : cc * 256 + (s + 1) * 128],
                                    rhs=vsl,
                                    start=(bh == 0 and g == 0 and cc == 0 and s == 0),
                                    stop=(
                                        bh == PAIRS_PER_CORE - 1
                                        and g == NGROUP - 2
                                        and cc == G - 1
                                        and s == 1
                                    ),
                                    skip_group_check=True,
                                )
                        snap = st_pool.tile([128, 2 * OW], fp8, tag="snap")
                        nc.vector.tensor_copy(
                            snap[:].rearrange("p (s w) -> p s w", s=2),
                            psKV[:].rearrange("p (s w) -> p s w", s=2)[:, :, 0:OW],
                        )
                        prev_snap = snap

                # ---- evict num^T for the superblock, ship fp32; the last
                # superblock evicts per half so the tail chain starts early ----
                if sp < NSUPER - 1:
                    outN_sb = on_pool.tile([OW, 512], f32, tag="outN")
                    nc.scalar.copy(outN_sb[:], psN[:])
                    nc.sync.dma_start(
                        out_d.ap()[bh, :, sp * 512 : (sp + 1) * 512], outN_sb[:]
                    )
                else:
                    for hi in range(2):
                        outH_sb = on_pool.tile([OW, 256], f32, tag="outH")
                        nc.scalar.copy(outH_sb[:], psN[:, hi * 256 : (hi + 1) * 256])
                        nc.sync.dma_start(
                            out_d.ap()[
                                bh, :, (sp * 2 + hi) * 256 : (sp * 2 + hi + 1) * 256
                            ],
                            outH_sb[:],
                        )

    nc.compile()
    return nc


def _get_program():
    global _CACHED_NC
    if _CACHED_NC is None:
        _CACHED_NC = _build_program()
    return _CACHED_NC


def _pack_inputs(query, key_t, value, projection_matrix):
    """Host-side sharding + layout packing. Returns list of 8 in_maps."""
    import ml_dtypes

    bf16 = ml_dtypes.bfloat16
    q = np.asarray(query, dtype=np.float32)
    k = np.asarray(key_t, dtype=np.float32)
    v = np.asarray(value, dtype=np.float32)
    proj = np.asarray(projection_matrix, dtype=np.float32)

    pt = (proj.T * RATIO).astype(np.float32)  # [D, M]
    ptqk = np.zeros((128, M), bf16)
    ptqk[0:64] = pt.astype(bf16)
    ptqk[64:128] = pt.astype(bf16)
    ptkz = np.zeros((128, M), bf16)
    ptkz[64:128] = pt.astype(bf16)
    tri = np.triu(np.ones((128, 128), np.float32))
    ones = np.ones((128, 128), np.float32)
    maskd = np.concatenate([tri, ones, tri], axis=1).astype(bf16)

    in_maps = []
    for core in range(N_CORES):
        qkt = np.empty((PAIRS_PER_CORE, 128, L), bf16)
        vaug = np.zeros((PAIRS_PER_CORE, 128, NCHUNK, VW), bf16)
        for local in range(PAIRS_PER_CORE):
            p = core * PAIRS_PER_CORE + local
            b, h = p // H, p % H
            qkt[local, 0:64] = q[b, :, h, :].T.astype(bf16)
            qkt[local, 64:128] = k[b, :, h, :].T.astype(bf16)
            vb = v[b, :, h, :].reshape(NCHUNK, C, D).transpose(1, 0, 2)  # [128,32,64]
            vaug[local, :, :, 0:D] = vb.astype(bf16)
            # 1/16 (not 1.0): keeps the running key-sum within fp8e4m3's 448
            # max when the KV snapshot quantizes to fp8; den rescaled on host
            vaug[local, :, :, D] = 1.0 / 16.0
        in_maps.append(
            {
                "qkt": qkt,
                "vaug": vaug.reshape(PAIRS_PER_CORE, 128, NCHUNK * VW),
                "ptqk": ptqk,
                "ptkz": ptkz,
                "maskd": maskd,
            }
        )
    return in_maps


def _unpack_outputs(results):
    """results: 8 dicts with 'out' [2, 65, 4096] fp32 -> [B, L, H, D]."""
    out = np.empty((B, L, H, D), np.float32)
    for core in range(N_CORES):
        arr = np.asarray(results[core]["out"], np.float32)
        for local in range(PAIRS_PER_CORE):
            p = core * PAIRS_PER_CORE + local
            b, h = p // H, p % H
            num = arr[local, 0:D, :]          # [64, L]
            den = arr[local, D, :] * 16.0     # [L] (ones column is 1/16)
            den = np.where(den <= 0.0, 1.0, den)
            out[b, :, h, :] = (num / den).T
    return out


def _ensure_axon_hooks():
    """Provide antenv.axon_hooks (NTFF profile hook) if the image lacks it."""
    import importlib

    try:
        importlib.import_module("antenv.axon_hooks")
        return
    except ImportError:
        pass
    try:
        import contextlib
        import ctypes
        import types

        so_path = "/opt/axon/libaxon_pjrt.so"
        if not os.path.exists(so_path):
            return
        lib = ctypes.CDLL(so_path)
        if not hasattr(lib, "axon_start_nrt_profile"):
            return
        lib.axon_start_nrt_profile.argtypes = [
            ctypes.POINTER(ctypes.c_int64),
            ctypes.c_size_t,
        ]
        lib.axon_start_nrt_profile.restype = ctypes.c_int64
        lib.axon_stop_nrt_profile.argtypes = [ctypes.c_char_p]
        lib.axon_stop_nrt_profile.restype = ctypes.c_int64

        @contextlib.contextmanager
        def _hook(output_dir, device_ids):
            import jax

            jax.devices()
            if device_ids:
                ids = (ctypes.c_int64 * len(device_ids))(*device_ids)
                rc = lib.axon_start_nrt_profile(ids, len(device_ids))
            else:
                rc = lib.axon_start_nrt_profile(None, 0)
            if rc != 0:
                raise RuntimeError(f"axon_start_nrt_profile rc={rc}")
            try:
                yield
            finally:
                n = lib.axon_stop_nrt_profile(str(output_dir).encode())
                print(f"profile: {n} file(s) written to {output_dir}", file=sys.stderr)

        mod = types.ModuleType("antenv.axon_hooks")
        mod._hook = _hook
        mod.get_axon_ntff_profile_hook = lambda: mod._hook
        mod.set_axon_ntff_profile_hook = lambda h: setattr(mod, "_hook", h)
        import antenv

        sys.modules["antenv.axon_hooks"] = mod
        antenv.axon_hooks = mod
    except Exception:
        pass


def kernel(query, key_t, value, projection_matrix):
    from concourse import bass_utils

    _ensure_axon_hooks()

    in_maps = _pack_inputs(query, key_t, value, projection_matrix)
    nc = _get_program()
    res = bass_utils.run_bass_kernel_spmd(
        nc,
        in_maps,
        core_ids=list(range(N_CORES)),
        trace=bool(int(os.environ.get("KERNEL_TRACE", "0"))),
    )
    out = _unpack_outputs(res.results)
    if res.exec_time_ns is not None:
        kernel.last_exec_time_ns = res.exec_time_ns
    kernel.last_results = res
    return out


kernel.last_exec_time_ns = None
kernel.last_results = None


# revision 43
# speedup vs baseline: 1.0658x; 1.0658x over previous
"""Trainium2 Bass kernel for Performer-style (FAVOR+) causal linear attention, v2.

Per (b,h): qp = relu(q @ ptT/sqrt(M)), kp likewise; causal chunked linear
attention, chunk C=128, groups of G=2 chunks, superblocks of 2 groups.

v2 layout/structure vs v1 (74.7us vs 81.4us measured):
  - Features per 256-token group: 4 N=256 matmuls with q-features on PE rows
    0:63 and k-features on rows 64:127 (base_partition row tiling ->
    concurrent pairs), emitted ONE GROUP AHEAD (software pipeline) so the
    relu eviction never gates the A-matmuls; one merged relu ACTIVATE
    [128, 1024] on ACT evicts all four slices (amortizes the 352-cyc per-op
    overhead and keeps the A-matmul dependency off the congested DVE queue).
  - num/den accumulate TRANSPOSED: psN^T[d, i] (65 partitions) so the
    stationary operands of the num matmuls are 65-column tiles (vaug chunks,
    KV snapshot slices) -> LDWEIGHTS 54ns instead of 113ns.
  - psA layout [diag0 | cross | diag1]; only the two diagonal blocks get the
    triangular mask (one strided 3D-AP DVE tensor_tensor); the cross block is
    a plain DVE convert-copy (it is multiplied by ones in v1).
  - psN evicted once per superblock [65, 512] (ACT), DMA'd straight out
    per superblock (no tail batch).
  - KV state in PSUM fp32 as in v1; snapshot once per group via one strided
    DVE copy; the last group's dead KV update/psP/snapshot are skipped.
Sharding: B*H = 16 pairs, 2 per core on 8 cores; division on host.
"""

import os
import sys

import numpy as np

sys.path.insert(0, "/opt/trn_rl_repo")

B, L, H, D, M = 2, 4096, 8, 64, 256
C = 128             # chunk length
NCHUNK = L // C     # 32
G = 2               # chunks per group
NGROUP = NCHUNK // G            # 16
NSUPER = NGROUP // 2            # 8 superblocks of 512 tokens
PAIRS_PER_CORE = 2
N_CORES = 8
VW = 72             # padded v_aug chunk width (64 v + 1 ones + 7 pad)
OW = 65             # 64 num + den
KVW = 72            # psKV slice stride
RATIO = 1.0 / np.sqrt(np.float32(M))

_CACHED_NC = None


def _build_program():
    import concourse.tile as tile
    from concourse import bacc, mybir
    from contextlib import ExitStack

    f32 = mybir.dt.float32
    bf16 = mybir.dt.bfloat16
    fp8 = mybir.dt.float8e4

    nc = bacc.Bacc("TRN2", target_bir_lowering=False, debug=False)

    qkt_d = nc.dram_tensor("qkt", [PAIRS_PER_CORE, 128, L], bf16, kind="ExternalInput")
    vaug_d = nc.dram_tensor(
        "vaug", [PAIRS_PER_CORE, 128, NCHUNK * VW], bf16, kind="ExternalInput"
    )
    ptqk_d = nc.dram_tensor("ptqk", [128, M], bf16, kind="ExternalInput")
    ptkz_d = nc.dram_tensor("ptkz", [128, M], bf16, kind="ExternalInput")
    maskd_d = nc.dram_tensor("maskd", [128, 3 * C], bf16, kind="ExternalInput")
    out_d = nc.dram_tensor(
        "out", [PAIRS_PER_CORE, OW, L], f32, kind="ExternalOutput"
    )

    with tile.TileContext(nc) as tc, ExitStack() as ctx:
        const_pool = ctx.enter_context(tc.tile_pool(name="const", bufs=1))
        io_pool = ctx.enter_context(tc.tile_pool(name="io", bufs=2))
        feat_pool = ctx.enter_context(tc.tile_pool(name="feat", bufs=2))
        kp_pool = ctx.enter_context(tc.tile_pool(name="kp", bufs=2))
        am_pool = ctx.enter_context(tc.tile_pool(name="am", bufs=2))
        st_pool = ctx.enter_context(tc.tile_pool(name="st", bufs=3))
        on_pool = ctx.enter_context(tc.tile_pool(name="on", bufs=2))

        ps_qk = ctx.enter_context(tc.tile_pool(name="psqk", bufs=2, space="PSUM"))
        ps_p = ctx.enter_context(tc.tile_pool(name="psp", bufs=1, space="PSUM"))
        ps_a = ctx.enter_context(tc.tile_pool(name="psa", bufs=1, space="PSUM"))
        ps_n = ctx.enter_context(tc.tile_pool(name="psn", bufs=1, space="PSUM"))
        ps_kv = ctx.enter_context(tc.tile_pool(name="pskv", bufs=1, space="PSUM"))

        ptqk_sb = const_pool.tile([128, M], bf16)
        nc.scalar.dma_start(ptqk_sb[:], ptqk_d.ap())
        ptkz_sb = const_pool.tile([128, M], bf16)
        maskd_sb = const_pool.tile([128, 3 * C], bf16)

        # Persistent fp32 KV state for both pairs in one PSUM bank; single
        # accumulation group over the whole kernel, per-byte first-touch.
        psKV_all = ps_kv.tile([128, PAIRS_PER_CORE * 2 * KVW], f32, tag="psKV")

        for bh in range(PAIRS_PER_CORE):
            qkt_sb = io_pool.tile([128, L], bf16, tag="qkt_sb")
            vaug_sb = io_pool.tile([128, NCHUNK * VW], bf16, tag="vaug_sb")
            if bh == 0:
                qsplits = [0, 256, 512, 1024, 2048, 3072, L]
                vsplits = [0, 2 * VW, 8 * VW, 16 * VW, NCHUNK * VW]
            else:
                qsplits = [0, 2048, L]
                vsplits = [0, NCHUNK * VW]
            nc.sync.dma_start(
                qkt_sb[:, 0 : qsplits[1]], qkt_d.ap()[bh, :, 0 : qsplits[1]]
            )
            if bh == 0:
                nc.scalar.dma_start(ptkz_sb[:], ptkz_d.ap())
            (nc.scalar if bh == 0 else nc.sync).dma_start(
                vaug_sb[:, 0 : vsplits[1]], vaug_d.ap()[bh, :, 0 : vsplits[1]]
            )
            if bh == 0:
                nc.sync.dma_start(maskd_sb[:], maskd_d.ap())
            for a, b in zip(qsplits[1:-1], qsplits[2:]):
                nc.sync.dma_start(qkt_sb[:, a:b], qkt_d.ap()[bh, :, a:b])
            for a, b in zip(vsplits[1:-1], vsplits[2:]):
                # keep the sync queue free for qkt chunks during the ramp
                (nc.scalar if bh == 0 else nc.sync).dma_start(
                    vaug_sb[:, a:b], vaug_d.ap()[bh, :, a:b]
                )

            psKV = psKV_all[:, bh * 2 * KVW : (bh + 1) * 2 * KVW]
            prev_snap = None

            def emit_features(g):
                """Feature matmuls + split relu eviction for group g.

                q-features on PE rows 0:63, k-features on rows 64:127
                (concurrent row-groups); relu split DVE (k) / ACT (q).
                Returns the bf16 [128, 1024] tile [q_s0|q_s1|k_s0|k_s1].
                """
                tk = slice(g * 256, (g + 1) * 256)
                psQK = ps_qk.tile([128, 4 * 256], f32, tag="psQK")
                for s in range(2):
                    nc.tensor.matmul(
                        psQK[:, s * 256 : (s + 1) * 256],
                        lhsT=ptqk_sb[0:64, s * 128 : (s + 1) * 128],
                        rhs=qkt_sb[0:64, tk],
                        start=True,
                        stop=True,
                    )
                    nc.tensor.matmul(
                        psQK[:, (2 + s) * 256 : (3 + s) * 256],
                        lhsT=ptqk_sb[64:128, s * 128 : (s + 1) * 128],
                        rhs=qkt_sb[64:128, tk],
                        start=True,
                        stop=True,
                    )
                t = feat_pool.tile([128, 4 * 256], fp8, tag="qkpT")
                nc.scalar.activation(
                    t[:], psQK[:], mybir.ActivationFunctionType.Relu
                )
                return t

            feat_next = emit_features(0)

            for sp in range(NSUPER):
                psN = ps_n.tile([OW, 512], f32, tag="psN")

                for gi in range(2):
                    g = sp * 2 + gi
                    c0 = g * G          # chunk indices
                    toff = 0            # token offset within feature block
                    ncol = gi * 256     # psN half

                    qkpT_sb = feat_next
                    if g < NGROUP - 1:
                        feat_next = emit_features(g + 1)

                    def qpT(s, off, width=C):
                        return qkpT_sb[:, s * 256 + off : s * 256 + off + width]

                    def kpT(s, off, width=C):
                        return qkpT_sb[
                            :, 512 + s * 256 + off : 512 + s * 256 + off + width
                        ]

                    v0 = vaug_sb[:, c0 * VW : c0 * VW + OW]
                    v1 = vaug_sb[:, (c0 + 1) * VW : (c0 + 1) * VW + OW]

                    # ---- kp natural (for KV update): 2 N=256 matmuls; the
                    # last group never uses its KV update -> skip ----
                    if g < NGROUP - 1:
                        psP = ps_p.tile([128, 512], f32, tag="psP")
                        for cc in range(G):
                            nc.tensor.matmul(
                                psP[:, cc * 256 : (cc + 1) * 256],
                                lhsT=qkt_sb[:, (c0 + cc) * C : (c0 + cc + 1) * C],
                                rhs=ptkz_sb[:],
                                start=True,
                                stop=True,
                            )
                        kp_sb = kp_pool.tile([128, 512], bf16, tag="kp_sb")
                        nc.vector.tensor_scalar(
                            kp_sb[:], psP[:], 0.0, None, mybir.AluOpType.max
                        )

                    # ---- A^T blocks [diag0 | cross | diag1]: fp8 DoubleRow,
                    # both feature slices (K=256) packed 2-per-cell ----
                    qpT3 = qkpT_sb[:, 0:512].rearrange("p (s w) -> p s w", s=2)
                    kpT3 = qkpT_sb[:, 512:1024].rearrange("p (s w) -> p s w", s=2)
                    psA = ps_a.tile([128, 3 * C], f32, tag="psA")
                    nc.tensor.matmul(
                        psA[:, 0 : 2 * C],
                        lhsT=kpT3[:, :, 0:C],
                        rhs=qpT3,
                        start=True,
                        stop=True,
                        perf_mode=mybir.MatmulPerfMode.DoubleRow,
                        skip_group_check=True,
                    )
                    nc.tensor.matmul(
                        psA[:, 2 * C : 3 * C],
                        lhsT=kpT3[:, :, C : 2 * C],
                        rhs=qpT3[:, :, C : 2 * C],
                        start=True,
                        stop=True,
                        perf_mode=mybir.MatmulPerfMode.DoubleRow,
                        skip_group_check=True,
                    )
                    amask_sb = am_pool.tile([128, 3 * C], bf16, tag="amask")
                    # one [tri|ones|tri] masked multiply: one DVE op beats
                    # diag-TT + cross-copy (per-op overhead); keeps DVE's
                    # per-group total under the group period
                    nc.vector.tensor_mul(amask_sb[:], psA[:], maskd_sb[:])

                    # ---- num^T (+den row 64): inter first, then masked ----
                    if g > 0:
                        for s in range(2):
                            nc.tensor.matmul(
                                psN[:, ncol : ncol + 256],
                                lhsT=prev_snap[:, s * OW : (s + 1) * OW],
                                rhs=qpT(s, toff, width=256),
                                start=(s == 0),
                                stop=False,
                                skip_group_check=True,
                            )
                    nc.tensor.matmul(
                        psN[:, ncol : ncol + 256],
                        lhsT=v0,
                        rhs=amask_sb[:, 0 : 2 * C],
                        start=(g == 0),
                        stop=False,
                        skip_group_check=True,
                    )
                    nc.tensor.matmul(
                        psN[:, ncol + C : ncol + 2 * C],
                        lhsT=v1,
                        rhs=amask_sb[:, 2 * C : 3 * C],
                        start=False,
                        stop=True,
                        skip_group_check=True,
                    )

                    # ---- KV state accumulation (after snapshot below of the
                    # previous group has read the old state); the last group's
                    # update feeds nothing -> skipped entirely ----
                    if g < NGROUP - 1:
                        for cc in range(G):
                            vsl = vaug_sb[:, (c0 + cc) * VW : (c0 + cc) * VW + OW]
                            for s in range(2):
                                nc.tensor.matmul(
                                    psKV[:, s * KVW : s * KVW + OW],
                                    lhsT=kp_sb[:, cc * 256 + s * 128 : cc * 256 + (s + 1) * 128],
                                    rhs=vsl,
                                    start=(bh == 0 and g == 0 and cc == 0 and s == 0),
                                    stop=(
                                        bh == PAIRS_PER_CORE - 1
                                        and g == NGROUP - 2
                                        and cc == G - 1
                                        and s == 1
                                    ),
                                    skip_group_check=True,
                                )
                        snap = st_pool.tile([128, 2 * OW], fp8, tag="snap")
                        nc.vector.tensor_copy(
                            snap[:].rearrange("p (s w) -> p s w", s=2),
                            psKV[:].rearrange("p (s w) -> p s w", s=2)[:, :, 0:OW],
                        )
                        prev_snap = snap

                # ---- evict num^T for the superblock, ship fp32; the last
                # superblock evicts per half so the tail chain starts early ----
                if sp < NSUPER - 1:
                    outN_sb = on_pool.tile([OW, 512], f32, tag="outN")
                    nc.scalar.copy(outN_sb[:], psN[:])
                    nc.sync.dma_start(
                        out_d.ap()[bh, :, sp * 512 : (sp + 1) * 512], outN_sb[:]
                    )
                else:
                    for hi in range(2):
                        outH_sb = on_pool.tile([OW, 256], f32, tag="outH")
                        nc.scalar.copy(outH_sb[:], psN[:, hi * 256 : (hi + 1) * 256])
                        nc.sync.dma_start(
                            out_d.ap()[
                                bh, :, (sp * 2 + hi) * 256 : (sp * 2 + hi + 1) * 256
                            ],
                            outH_sb[:],
                        )

    nc.compile()
    return nc


def _get_program():
    global _CACHED_NC
    if _CACHED_NC is None:
        _CACHED_NC = _build_program()
    return _CACHED_NC


def _pack_inputs(query, key_t, value, projection_matrix):
    """Host-side sharding + layout packing. Returns list of 8 in_maps."""
    import ml_dtypes

    bf16 = ml_dtypes.bfloat16
    q = np.asarray(query, dtype=np.float32)
    k = np.asarray(key_t, dtype=np.float32)
    v = np.asarray(value, dtype=np.float32)
    proj = np.asarray(projection_matrix, dtype=np.float32)

    pt = (proj.T * RATIO).astype(np.float32)  # [D, M]
    ptqk = np.zeros((128, M), bf16)
    ptqk[0:64] = pt.astype(bf16)
    ptqk[64:128] = pt.astype(bf16)
    ptkz = np.zeros((128, M), bf16)
    ptkz[64:128] = pt.astype(bf16)
    tri = np.triu(np.ones((128, 128), np.float32))
    ones = np.ones((128, 128), np.float32)
    maskd = np.concatenate([tri, ones, tri], axis=1).astype(bf16)

    in_maps = []
    for core in range(N_CORES):
        qkt = np.empty((PAIRS_PER_CORE, 128, L), bf16)
        vaug = np.zeros((PAIRS_PER_CORE, 128, NCHUNK, VW), bf16)
        for local in range(PAIRS_PER_CORE):
            p = core * PAIRS_PER_CORE + local
            b, h = p // H, p % H
            qkt[local, 0:64] = q[b, :, h, :].T.astype(bf16)
            qkt[local, 64:128] = k[b, :, h, :].T.astype(bf16)
            vb = v[b, :, h, :].reshape(NCHUNK, C, D).transpose(1, 0, 2)  # [128,32,64]
            vaug[local, :, :, 0:D] = vb.astype(bf16)
            # 1/16 (not 1.0): keeps the running key-sum within fp8e4m3's 448
            # max when the KV snapshot quantizes to fp8; den rescaled on host
            vaug[local, :, :, D] = 1.0 / 16.0
        in_maps.append(
            {
                "qkt": qkt,
                "vaug": vaug.reshape(PAIRS_PER_CORE, 128, NCHUNK * VW),
                "ptqk": ptqk,
                "ptkz": ptkz,
                "maskd": maskd,
            }
        )
    return in_maps


def _unpack_outputs(results):
    """results: 8 dicts with 'out' [2, 65, 4096] fp32 -> [B, L, H, D]."""
    out = np.empty((B, L, H, D), np.float32)
    for core in range(N_CORES):
        arr = np.asarray(results[core]["out"], np.float32)
        for local in range(PAIRS_PER_CORE):
            p = core * PAIRS_PER_CORE + local
            b, h = p // H, p % H
            num = arr[local, 0:D, :]          # [64, L]
            den = arr[local, D, :] * 16.0     # [L] (ones column is 1/16)
            den = np.where(den <= 0.0, 1.0, den)
            out[b, :, h, :] = (num / den).T
    return out


def _ensure_axon_hooks():
    """Provide antenv.axon_hooks (NTFF profile hook) if the image lacks it."""
    import importlib

    try:
        importlib.import_module("antenv.axon_hooks")
        return
    except ImportError:
        pass
    try:
        import contextlib
        import ctypes
        import types

        so_path = "/opt/axon/libaxon_pjrt.so"
        if not os.path.exists(so_path):
            return
        lib = ctypes.CDLL(so_path)
        if not hasattr(lib, "axon_start_nrt_profile"):
            return
        lib.axon_start_nrt_profile.argtypes = [
            ctypes.POINTER(ctypes.c_int64),
            ctypes.c_size_t,
        ]
        lib.axon_start_nrt_profile.restype = ctypes.c_int64
        lib.axon_stop_nrt_profile.argtypes = [ctypes.c_char_p]
        lib.axon_stop_nrt_profile.restype = ctypes.c_int64

        @contextlib.contextmanager
        def _hook(output_dir, device_ids):
            import jax

            jax.devices()
            if device_ids:
                ids = (ctypes.c_int64 * len(device_ids))(*device_ids)
                rc = lib.axon_start_nrt_profile(ids, len(device_ids))
            else:
                rc = lib.axon_start_nrt_profile(None, 0)
            if rc != 0:
                raise RuntimeError(f"axon_start_nrt_profile rc={rc}")
            try:
                yield
            finally:
                n = lib.axon_stop_nrt_profile(str(output_dir).encode())
                print(f"profile: {n} file(s) written to {output_dir}", file=sys.stderr)

        mod = types.ModuleType("antenv.axon_hooks")
        mod._hook = _hook
        mod.get_axon_ntff_profile_hook = lambda: mod._hook
        mod.set_axon_ntff_profile_hook = lambda h: setattr(mod, "_hook", h)
        import antenv

        sys.modules["antenv.axon_hooks"] = mod
        antenv.axon_hooks = mod
    except Exception:
        pass


def kernel(query, key_t, value, projection_matrix):
    from concourse import bass_utils

    _ensure_axon_hooks()

    in_maps = _pack_inputs(query, key_t, value, projection_matrix)
    nc = _get_program()
    res = bass_utils.run_bass_kernel_spmd(
        nc,
        in_maps,
        core_ids=list(range(N_CORES)),
        trace=bool(int(os.environ.get("KERNEL_TRACE", "0"))),
    )
    out = _unpack_outputs(res.results)
    if res.exec_time_ns is not None:
        kernel.last_exec_time_ns = res.exec_time_ns
    kernel.last_results = res
    return out


kernel.last_exec_time_ns = None
kernel.last_results = None
